# revision 18
# baseline (speedup 1.0000x reference)
"""Trainium2 Bass kernel for nn_GCN_32289564131895 (gnn_message_passing).

8 NeuronCores, node-sharded (512 rows/core), weights replicated, on-device
collectives (AllGather/AllReduce). Key ideas:

- Dense masked adjacency: top-32 selection becomes a per-row threshold tau
  (exact: per-1024-block top-32 candidates always cover the row top-32, the
  merge of 4x32 candidates yields the exact 32nd max). W = adj * (adj>=tau).
  The dinv[idx] gather becomes a column scale; both gather-einsums become
  dense matmuls against W / W.T. No gathers or scatters anywhere.
- mm1/mm2 in true fp32 on PE (top-k boundary-exact: gaps between the 32nd
  and 33rd order stats go down to 4e-7 while fp32r noise is ~3e-5, so fp32r
  would flip ~10% of rows; fp32 flips none).
- Everything after selection runs in fp32r (1 cyc/row vs fp32's 4).
- adj and W.T spill through DRAM to fit the 192KB/partition SBUF budget;
  block-candidate top-k overlaps mm2 on the DVE.
"""

import sys

for _p in ("/opt/trn_rl_repo", "/root/.axon_site/_ro/trn_rl_repo"):
    if _p not in sys.path:
        sys.path.insert(0, _p)

from contextlib import ExitStack

import numpy as np

import concourse.bass as bass
import concourse.mybir as mybir
import concourse.tile as tile
from concourse import bacc
from concourse.bass_utils import run_bass_kernel_spmd
from concourse.masks import make_identity

dt = mybir.dt
AF = mybir.ActivationFunctionType
ALU = mybir.AluOpType

N_CORES = 8
N = 4096
D = 4096
H_MLP = 1024
HID = 256
OUT = 256
BN_EPS = 1e-5

KIN = D + 4 + 1001        # 5101
KIN_PAD = 5120
ROWS = N // N_CORES       # 512
RT = ROWS // 128          # 4
KT1 = KIN_PAD // 128      # 40
NT = H_MLP // 128         # 8
JT = N // 128             # 32
CT = HID // 128           # 2
FT = D // 128             # 32
JQ = 4
JQW = N // JQ             # 1024

TRACE = False
DEBUG_DUMPS = False
LAST_INFO = {}
_CACHED_NC = None


def _build():
    nc = bacc.Bacc(None, target_bir_lowering=False)
    f32 = dt.float32
    f32r = dt.float32r

    fp16 = dt.float16
    at_h = nc.declare_dram_parameter("at_h", [KIN_PAD, ROWS], fp16, isOutput=False)
    at_l = nc.declare_dram_parameter("at_l", [KIN_PAD, ROWS], fp16, isOutput=False)
    w1h = nc.declare_dram_parameter("w1h", [KIN_PAD, H_MLP], fp16, isOutput=False)
    w1l = nc.declare_dram_parameter("w1l", [KIN_PAD, H_MLP], fp16, isOutput=False)
    b1 = nc.declare_dram_parameter("b1", [H_MLP], f32, isOutput=False)
    w2h = nc.declare_dram_parameter("w2h", [H_MLP, N], fp16, isOutput=False)
    w2l = nc.declare_dram_parameter("w2l", [H_MLP, N], fp16, isOutput=False)
    b2h = nc.declare_dram_parameter("b2h", [N], fp16, isOutput=False)
    b2l = nc.declare_dram_parameter("b2l", [N], fp16, isOutput=False)
    nodet = nc.declare_dram_parameter("nodet", [D, ROWS], f32r, isOutput=False)
    cw1 = nc.declare_dram_parameter("cw1", [D, HID], f32r, isOutput=False)
    b1c = nc.declare_dram_parameter("b1c", [HID], f32, isOutput=False)
    cw2 = nc.declare_dram_parameter("cw2", [HID, OUT], f32r, isOutput=False)
    b2c = nc.declare_dram_parameter("b2c", [OUT], f32, isOutput=False)
    gamma = nc.declare_dram_parameter("gamma", [HID], f32, isOutput=False)
    beta = nc.declare_dram_parameter("beta", [HID], f32, isOutput=False)
    out = nc.declare_dram_parameter("out", [OUT, ROWS], f32, isOutput=True)
    if DEBUG_DUMPS:
        h_dbg = nc.declare_dram_parameter("h_dbg", [H_MLP, ROWS], f32, isOutput=True)
        adj_dbg = nc.declare_dram_parameter("adj_dbg", [ROWS, N], f32, isOutput=True)

    # internal DRAM
    adj_dram = nc.dram_tensor("adj_dram", [ROWS, N], f32)
    p_shard = nc.dram_tensor("p_shard", [ROWS, HID], f32r)
    p_full = nc.dram_tensor("p_full", [N, HID], f32r, addr_space="Shared")
    deg_shard = nc.dram_tensor("deg_shard", [ROWS], f32)
    deg_full = nc.dram_tensor("deg_full", [N], f32, addr_space="Shared")
    stats_loc = nc.dram_tensor("stats_loc", [4 * 128], f32)
    stats_red = nc.dram_tensor("stats_red", [8, 4 * 128], f32, addr_space="Shared")
    q_shard = nc.dram_tensor("q_shard", [ROWS, OUT], f32r)
    q_full = nc.dram_tensor("q_full", [N, OUT], f32r, addr_space="Shared")

    GRP = [list(range(N_CORES))]

    with tile.TileContext(nc) as tc:
        with (
            tc.tile_pool(name="const", bufs=1) as const,
            tc.tile_pool(name="hold", bufs=1) as hold,
            tc.tile_pool(name="wstage", bufs=4) as wstage,
        ):
            sstack = ExitStack()
            st3 = sstack.enter_context(tc.tile_pool(name="st3", bufs=3))
            st2 = sstack.enter_context(tc.tile_pool(name="st2", bufs=2))
            stage = sstack.enter_context(tc.tile_pool(name="stage", bufs=2))
            # ---------------- constants ----------------
            b1_sb = const.tile([128, NT], f32, tag="b1")
            nc.sync.dma_start(b1_sb[:], b1.rearrange("(t p) -> p t", p=128))
            b2h_sb = const.tile([1, N], fp16, tag="b2h")
            nc.sync.dma_start(b2h_sb[:], b2h.rearrange("(o j) -> o j", o=1))
            b2l_sb = const.tile([1, N], fp16, tag="b2l")
            nc.sync.dma_start(b2l_sb[:], b2l.rearrange("(o j) -> o j", o=1))
            ones16 = const.tile([1, 128], fp16, tag="ones16")
            nc.vector.memset(ones16[:], 1.0)
            b1c_sb = const.tile([128, CT], f32, tag="b1c")
            nc.sync.dma_start(b1c_sb[:], b1c.rearrange("(t p) -> p t", p=128))
            b2c_sb = const.tile([128, CT], f32, tag="b2c")
            nc.sync.dma_start(b2c_sb[:], b2c.rearrange("(t p) -> p t", p=128))
            gam_sb = const.tile([128, CT], f32, tag="gam")
            nc.sync.dma_start(gam_sb[:], gamma.rearrange("(t p) -> p t", p=128))
            bet_sb = const.tile([128, CT], f32, tag="bet")
            nc.sync.dma_start(bet_sb[:], beta.rearrange("(t p) -> p t", p=128))
            cw2_sb = const.tile([128, CT, OUT], f32r, tag="cw2")
            nc.sync.dma_start(cw2_sb[:], cw2.rearrange("(t p) c -> p t c", p=128))
            ones1 = const.tile([1, 128], f32, tag="ones1")
            nc.vector.memset(ones1[:], 1.0)
            ident = const.tile([128, 128], f32, tag="ident")
            make_identity(nc, ident[:])
            # per-(i,jq)-block top-32 candidate values
            btop = const.tile([128, RT, JQ, 32], f32, tag="btop")

            # ---------------- phase 0: P = node_emb @ conv_w1 (row shard) ----
            ps0 = ExitStack()
            pp = ps0.enter_context(tc.tile_pool(name="pp", bufs=4, space="PSUM"))
            psum_p = [pp.tile([128, HID], f32, tag="pp", name=f"psum_p{_i}")
                      for _i in range(RT)]
            for f in range(FT):
                nt_t = st3.tile([128, ROWS], f32r, tag="nt")
                nc.sync.dma_start(nt_t[:], nodet[f * 128:(f + 1) * 128, :])
                cw1_t = st3.tile([128, HID], f32r, tag="cw1")
                nc.sync.dma_start(cw1_t[:], cw1[f * 128:(f + 1) * 128, :])
                for i in range(RT):
                    nc.tensor.matmul(
                        psum_p[i][:],
                        lhsT=nt_t[:, i * 128:(i + 1) * 128],
                        rhs=cw1_t[:],
                        start=(f == 0), stop=(f == FT - 1),
                    )
            p_sb = hold.tile([128, RT, HID], f32r, tag="io_small")
            for i in range(RT):
                nc.vector.tensor_copy(p_sb[:, i, :], psum_p[i][:])
            nc.sync.dma_start(p_shard.rearrange("(t p) c -> p t c", p=128), p_sb[:])
            nc.gpsimd.collective_compute(
                "AllGather", ALU.bypass, replica_groups=GRP,
                ins=[p_shard[:, :]], outs=[p_full[:, :]],
            )
            ps0.close()

            # ---------------- phase 1: mm1 (fp32): h.T[n, i] ----------------
            ps1 = ExitStack()
            ph = ps1.enter_context(tc.tile_pool(name="ph", bufs=8, space="PSUM"))
            psum_h = [ph.tile([128, ROWS], f32, tag="ph", name=f"psum_h{_i}")
                      for _i in range(NT)]
            for k in range(KT1):
                ath_t = st3.tile([128, ROWS], fp16, tag="ath")
                nc.sync.dma_start(ath_t[:], at_h[k * 128:(k + 1) * 128, :])
                atl_t = st3.tile([128, ROWS], fp16, tag="atl")
                nc.sync.dma_start(atl_t[:], at_l[k * 128:(k + 1) * 128, :])
                w1h_t = st2.tile([128, H_MLP], fp16, tag="w1h")
                nc.sync.dma_start(w1h_t[:], w1h[k * 128:(k + 1) * 128, :])
                w1l_t = st2.tile([128, H_MLP], fp16, tag="w1l")
                nc.sync.dma_start(w1l_t[:], w1l[k * 128:(k + 1) * 128, :])
                for n in range(NT):
                    nc.tensor.matmul(
                        psum_h[n][:],
                        lhsT=w1h_t[:, n * 128:(n + 1) * 128],
                        rhs=ath_t[:],
                        start=(k == 0), stop=False,
                    )
                    nc.tensor.matmul(
                        psum_h[n][:],
                        lhsT=w1h_t[:, n * 128:(n + 1) * 128],
                        rhs=atl_t[:],
                        start=False, stop=False,
                    )
                    nc.tensor.matmul(
                        psum_h[n][:],
                        lhsT=w1l_t[:, n * 128:(n + 1) * 128],
                        rhs=ath_t[:],
                        start=False, stop=(k == KT1 - 1),
                    )
            h_f = hold.tile([128, NT, ROWS], f32, tag="gr")  # shares w/ p_all
            hh_sb = hold.tile([128, NT, ROWS], fp16, tag="hh")
            hl_sb = hold.tile([128, NT, ROWS], fp16, tag="hl")
            for n in range(NT):
                hup = stage.tile([128, ROWS], f32, tag="hup")
                nc.scalar.activation(h_f[:, n, :], psum_h[n][:], AF.Relu,
                                     bias=b1_sb[:, n:n + 1], scale=1.0)
                nc.vector.tensor_copy(hh_sb[:, n, :], h_f[:, n, :])
                nc.vector.tensor_copy(hup[:], hh_sb[:, n, :])
                nc.vector.tensor_sub(hup[:], h_f[:, n, :], hup[:])
                nc.vector.tensor_copy(hl_sb[:, n, :], hup[:])
                if DEBUG_DUMPS:
                    nc.sync.dma_start(h_dbg[n * 128:(n + 1) * 128, :], h_f[:, n, :])

            ps1.close()

            # ------- phase 2: mm2 (fp32) + spill + block top-32 candidates ---
            ps2 = ExitStack()
            pa = ps2.enter_context(tc.tile_pool(name="pa", bufs=4, space="PSUM"))
            for jq in range(JQ):
                psum_a = [pa.tile([128, JQW], f32, tag="pa",
                                  name=f"psum_a{jq}_{_i}") for _i in range(RT)]
                for n in range(NT):
                    w2h_t = st2.tile([128, JQW], fp16, tag="w2h")
                    nc.sync.dma_start(
                        w2h_t[:], w2h[n * 128:(n + 1) * 128, jq * JQW:(jq + 1) * JQW])
                    w2l_t = st2.tile([128, JQW], fp16, tag="w2l")
                    nc.sync.dma_start(
                        w2l_t[:], w2l[n * 128:(n + 1) * 128, jq * JQW:(jq + 1) * JQW])
                    for i in range(RT):
                        for hh in range(JQW // 512):
                            nc.tensor.matmul(
                                psum_a[i][:, hh * 512:(hh + 1) * 512],
                                lhsT=hh_sb[:, n, i * 128:(i + 1) * 128],
                                rhs=w2h_t[:, hh * 512:(hh + 1) * 512],
                                start=(n == 0), stop=False,
                            )
                            nc.tensor.matmul(
                                psum_a[i][:, hh * 512:(hh + 1) * 512],
                                lhsT=hh_sb[:, n, i * 128:(i + 1) * 128],
                                rhs=w2l_t[:, hh * 512:(hh + 1) * 512],
                                start=False, stop=False,
                            )
                            nc.tensor.matmul(
                                psum_a[i][:, hh * 512:(hh + 1) * 512],
                                lhsT=hl_sb[:, n, i * 128:(i + 1) * 128],
                                rhs=w2h_t[:, hh * 512:(hh + 1) * 512],
                                start=False, stop=False,
                            )
                for i in range(RT):
                    for hh in range(JQW // 512):
                        nc.tensor.matmul(
                            psum_a[i][:, hh * 512:(hh + 1) * 512], lhsT=ones16[:],
                            rhs=b2h_sb[:, jq * JQW + hh * 512:jq * JQW + (hh + 1) * 512],
                            start=False, stop=False,
                        )
                        nc.tensor.matmul(
                            psum_a[i][:, hh * 512:(hh + 1) * 512], lhsT=ones16[:],
                            rhs=b2l_sb[:, jq * JQW + hh * 512:jq * JQW + (hh + 1) * 512],
                            start=False, stop=True,
                        )
                for i in range(RT):
                    ast = stage.tile([128, JQW], f32, tag="ast")
                    nc.scalar.activation(ast[:], psum_a[i][:], AF.Copy)
                    nc.sync.dma_start(
                        adj_dram[i * 128:(i + 1) * 128, jq * JQW:(jq + 1) * JQW],
                        ast[:])
                    if DEBUG_DUMPS:
                        nc.sync.dma_start(
                            adj_dbg[i * 128:(i + 1) * 128, jq * JQW:(jq + 1) * JQW],
                            ast[:])
                    # block top-32 candidates (max8/match_replace x4)
                    zb = stage.tile([128, JQW], f32, tag="zb")
                    m8 = wstage.tile([128, 8], f32, tag="m8")
                    nc.vector.max(out=m8[:], in_=ast[:])
                    nc.vector.tensor_copy(btop[:, i, jq, 0:8], m8[:])
                    nc.vector.match_replace(out=zb[:], in_to_replace=m8[:],
                                            in_values=ast[:], imm_value=-1e30)
                    for r in range(1, 4):
                        nc.vector.max(out=m8[:], in_=zb[:])
                        nc.vector.tensor_copy(btop[:, i, jq, 8 * r:8 * r + 8], m8[:])
                        nc.vector.match_replace(out=zb[:], in_to_replace=m8[:],
                                                in_values=zb[:], imm_value=-1e30)

            ps2.close()

            sstack.close()
            relp = ExitStack()
            rel = relp.enter_context(tc.tile_pool(name="rel", bufs=2))

            # ------- phase 3: merge candidates -> tau, top-32 values -> deg ----
            ps3 = ExitStack()
            pt = ps3.enter_context(tc.tile_pool(name="pt", bufs=4, space="PSUM"))
            deg_sb = const.tile([128, RT], f32, tag="deg")
            dinv_own = const.tile([128, RT], f32, tag="dinv_own")
            tmp1 = const.tile([128, RT], f32, tag="tmp1")
            tau_sb = const.tile([128, RT], f32, tag="tau")
            top32 = const.tile([128, RT, 32], f32, tag="top32")
            for i in range(RT):
                cand = btop[:, i, :, :]  # [128, JQ, 32] = 128 candidates
                z2 = wstage.tile([128, JQ * 32], f32, tag="z2")
                m8b = wstage.tile([128, 8], f32, tag="m8b")
                nc.vector.max(out=m8b[:], in_=cand)
                nc.vector.tensor_copy(top32[:, i, 0:8], m8b[:])
                nc.vector.match_replace(out=z2[:], in_to_replace=m8b[:],
                                        in_values=cand, imm_value=-1e30)
                for r in range(1, 4):
                    nc.vector.max(out=m8b[:], in_=z2[:])
                    nc.vector.tensor_copy(top32[:, i, 8 * r:8 * r + 8], m8b[:])
                    nc.vector.match_replace(out=z2[:], in_to_replace=m8b[:],
                                            in_values=z2[:], imm_value=-1e30)
                nc.vector.tensor_copy(tau_sb[:, i:i + 1], m8b[:, 7:8])
                # deg = sum of the exact top-32 values; dinv guarded rsqrt
                nc.vector.reduce_sum(deg_sb[:, i:i + 1], top32[:, i, :],
                                     axis=mybir.AxisListType.X)
                nc.vector.tensor_scalar_max(tmp1[:, i:i + 1], deg_sb[:, i:i + 1],
                                            1e-12)
                nc.scalar.activation(tmp1[:, i:i + 1], tmp1[:, i:i + 1], AF.Sqrt)
                nc.vector.reciprocal(tmp1[:, i:i + 1], tmp1[:, i:i + 1])
                nc.vector.tensor_scalar(dinv_own[:, i:i + 1], deg_sb[:, i:i + 1],
                                        0.0, None, op0=ALU.is_gt)
                nc.vector.tensor_mul(dinv_own[:, i:i + 1], dinv_own[:, i:i + 1],
                                     tmp1[:, i:i + 1])

            # deg AllGather as early as possible (hidden behind W build)
            nc.sync.dma_start(deg_shard.rearrange("(t p) -> p t", p=128), deg_sb[:])
            nc.gpsimd.collective_compute(
                "AllGather", ALU.bypass, replica_groups=GRP,
                ins=[deg_shard[:]], outs=[deg_full[:]],
            )

            wtd_sb = hold.tile([128, JT, ROWS], f32r, tag="wtd")
            for i in range(RT):
                a_i = rel.tile([128, N], f32, tag="arel")
                nc.sync.dma_start(a_i[:], adj_dram[i * 128:(i + 1) * 128, :])
                for jq in range(JQ):
                    mk = rel.tile([128, JQW], f32, tag="mkc")
                    sl = slice(jq * JQW, (jq + 1) * JQW)
                    nc.vector.tensor_scalar(mk[:], a_i[:, sl], tau_sb[:, i:i + 1],
                                            dinv_own[:, i:i + 1],
                                            op0=ALU.is_ge, op1=ALU.mult)
                    nc.gpsimd.tensor_mul(a_i[:, sl], a_i[:, sl], mk[:])
                for jt in range(JT):
                    pst = pt.tile([128, 128], f32, tag="pt")
                    nc.tensor.transpose(pst[:], a_i[:, jt * 128:(jt + 1) * 128],
                                        ident[:])
                    dst = wtd_sb[:, jt, i * 128:(i + 1) * 128]
                    if jt % 4 == 0:
                        nc.vector.tensor_copy(dst, pst[:])
                    else:
                        nc.scalar.activation(dst, pst[:], AF.Copy)

            ps3.close()
            relp.close()
            deg_all = const.tile([128, JT], f32, tag="deg_all")
            nc.sync.dma_start(deg_all[:], deg_full.rearrange("(t p) -> p t", p=128))
            dinv_all = const.tile([128, JT], f32, tag="dinv_all")
            tmp2 = const.tile([128, JT], f32, tag="tmp2")
            nc.vector.tensor_scalar_max(tmp2[:], deg_all[:], 1e-12)
            nc.scalar.activation(tmp2[:], tmp2[:], AF.Sqrt)
            nc.vector.reciprocal(tmp2[:], tmp2[:])
            nc.vector.tensor_scalar(dinv_all[:], deg_all[:], 0.0, None,
                                    op0=ALU.is_gt)
            nc.vector.tensor_mul(dinv_all[:], dinv_all[:], tmp2[:])

            # ---------------- phase 5: Pd = dinv_col * P (in place) ----------
            pd = hold.tile([128, JT, HID], f32r, tag="gr")  # reuses h_f slot
            nc.sync.dma_start(pd[:], p_full.rearrange("(t p) c -> p t c", p=128))
            for jt in range(JT):
                nc.scalar.activation(pd[:, jt, :], pd[:, jt, :], AF.Copy,
                                     scale=dinv_all[:, jt:jt + 1])

            # ---------------- phase 6: msg1.T ------------------------------
            ps6 = ExitStack()
            pm = ps6.enter_context(tc.tile_pool(name="pm", bufs=2, space="PSUM"))
            obt = hold.tile([128, CT, ROWS], f32, tag="obt")
            for ct in range(CT):
                psm = pm.tile([128, ROWS], f32, tag="pm")
                for jt in range(JT):
                    nc.tensor.matmul(
                        psm[:],
                        lhsT=pd[:, jt, ct * 128:(ct + 1) * 128],
                        rhs=wtd_sb[:, jt, :],
                        start=(jt == 0), stop=(jt == JT - 1),
                    )
                nc.vector.tensor_scalar(obt[:, ct, :], psm[:], b1c_sb[:, ct:ct + 1],
                                        None, op0=ALU.add)

            # ---------------- phase 7: BatchNorm (global stats) --------------
            sq = hold.tile([128, CT, ROWS], f32, tag="sq")
            nc.vector.tensor_mul(sq[:], obt[:], obt[:])
            st_sb = const.tile([128, 4], f32, tag="st")
            for ct in range(CT):
                nc.vector.reduce_sum(st_sb[:, ct:ct + 1], obt[:, ct, :],
                                     axis=mybir.AxisListType.X)
                nc.vector.reduce_sum(st_sb[:, 2 + ct:3 + ct], sq[:, ct, :],
                                     axis=mybir.AxisListType.X)
            nc.sync.dma_start(stats_loc.rearrange("(t p) -> p t", p=128), st_sb[:])
            nc.gpsimd.collective_compute(
                "AllGather", ALU.bypass, replica_groups=GRP,
                ins=[stats_loc[:]], outs=[stats_red[:, :]],
            )
            str8 = const.tile([128, 8, 4], f32, tag="str8")
            nc.sync.dma_start(str8[:], stats_red.rearrange("r (t p) -> p r t", p=128))
            str_sb = const.tile([128, 4], f32, tag="str")
            nc.vector.tensor_add(str_sb[:], str8[:, 0, :], str8[:, 1, :])
            for r in range(2, 8):
                nc.vector.tensor_add(str_sb[:], str_sb[:], str8[:, r, :])
            mean = const.tile([128, CT], f32, tag="mean")
            var = const.tile([128, CT], f32, tag="var")
            nc.vector.tensor_scalar_mul(mean[:], str_sb[:, 0:CT], 1.0 / N)
            nc.vector.tensor_scalar_mul(var[:], str_sb[:, CT:2 * CT], 1.0 / N)
            msq = const.tile([128, CT], f32, tag="msq")
            nc.vector.tensor_mul(msq[:], mean[:], mean[:])
            nc.vector.tensor_sub(var[:], var[:], msq[:])
            rstd = const.tile([128, CT], f32, tag="rstd")
            nc.vector.tensor_scalar_add(rstd[:], var[:], BN_EPS)
            nc.scalar.activation(rstd[:], rstd[:], AF.Sqrt)
            nc.vector.reciprocal(rstd[:], rstd[:])
            s_bn = const.tile([128, CT], f32, tag="s_bn")
            nc.vector.tensor_mul(s_bn[:], gam_sb[:], rstd[:])
            t_bn = const.tile([128, CT], f32, tag="t_bn")
            nc.vector.tensor_mul(t_bn[:], mean[:], s_bn[:])
            nc.vector.tensor_sub(t_bn[:], bet_sb[:], t_bn[:])
            obnt = hold.tile([128, CT, ROWS], f32r, tag="obnt")
            for ct in range(CT):
                nc.vector.tensor_scalar(sq[:, ct, :], obt[:, ct, :],
                                        s_bn[:, ct:ct + 1], t_bn[:, ct:ct + 1],
                                        op0=ALU.mult, op1=ALU.add)
                nc.scalar.activation(obnt[:, ct, :], sq[:, ct, :], AF.Relu)

            # ---------------- phase 8: Q = out_bn @ conv_w2 ------------------
            pq_pool = ps6.enter_context(tc.tile_pool(name="pqp", bufs=2, space="PSUM"))
            q_sb = hold.tile([128, RT, OUT], f32r, tag="io_small")
            for i in range(RT):
                psq = pq_pool.tile([128, OUT], f32, tag="pq")
                for ct in range(CT):
                    nc.tensor.matmul(
                        psq[:],
                        lhsT=obnt[:, ct, i * 128:(i + 1) * 128],
                        rhs=cw2_sb[:, ct, :],
                        start=(ct == 0), stop=(ct == CT - 1),
                    )
                nc.vector.tensor_copy(q_sb[:, i, :], psq[:])
            nc.sync.dma_start(q_shard.rearrange("(t p) c -> p t c", p=128), q_sb[:])
            nc.gpsimd.collective_compute(
                "AllGather", ALU.bypass, replica_groups=GRP,
                ins=[q_shard[:, :]], outs=[q_full[:, :]],
            )
            qd = hold.tile([128, JT, OUT], f32r, tag="gr")
            nc.sync.dma_start(qd[:], q_full.rearrange("(t p) c -> p t c", p=128))
            for jt in range(JT):
                nc.scalar.activation(qd[:, jt, :], qd[:, jt, :], AF.Copy,
                                     scale=dinv_all[:, jt:jt + 1])

            # ---------------- phase 9: out.T = msg2.T + b2c ------------------
            fsb = hold.tile([128, CT, ROWS], f32, tag="io_small")
            for ct in range(CT):
                psf = pm.tile([128, ROWS], f32, tag="pf")
                for jt in range(JT):
                    nc.tensor.matmul(
                        psf[:],
                        lhsT=qd[:, jt, ct * 128:(ct + 1) * 128],
                        rhs=wtd_sb[:, jt, :],
                        start=(jt == 0), stop=(jt == JT - 1),
                    )
                nc.vector.tensor_scalar(fsb[:, ct, :], psf[:], b2c_sb[:, ct:ct + 1],
                                        None, op0=ALU.add)
            nc.sync.dma_start(out.rearrange("(t p) i -> p t i", p=128), fsb[:])
            ps6.close()

    nc.compile()
    return nc


def _device_reset():
    """Tiny SPMD program to clear wedged device state after a crash."""
    nc = bacc.Bacc(None, target_bir_lowering=False)
    x = nc.declare_dram_parameter("x", [128, 128], dt.float32, isOutput=False)
    y = nc.declare_dram_parameter("y", [128, 128], dt.float32, isOutput=True)
    with tile.TileContext(nc) as tc:
        with tc.tile_pool(name="sb", bufs=1) as sb:
            t = sb.tile([128, 128], dt.float32, tag="t")
            nc.sync.dma_start(t[:], x[:, :])
            nc.vector.tensor_scalar_add(t[:], t[:], 1.0)
            nc.sync.dma_start(y[:, :], t[:])
    nc.compile()
    z = np.zeros((128, 128), np.float32)
    run_bass_kernel_spmd(nc, [{"x": z} for _ in range(N_CORES)],
                         list(range(N_CORES)))


def kernel(probs, bbox_coords, query_emb, node_emb,
           mlp_w1, mlp_b1, mlp_w2, mlp_b2,
           conv_w1, conv_b1, conv_w2, conv_b2,
           bn_gamma, bn_beta):
    global _CACHED_NC
    if _CACHED_NC is None:
        _CACHED_NC = _build()
    nc = _CACHED_NC

    f = np.float32

    def split_hl(x):
        hi = x.astype(np.float16)
        lo = (x - hi.astype(f)).astype(np.float16)
        return np.ascontiguousarray(hi), np.ascontiguousarray(lo)

    ew = np.concatenate([np.asarray(query_emb, f), np.asarray(probs, f),
                         np.asarray(bbox_coords, f)], axis=1)
    at_full = np.zeros((KIN_PAD, N), f)
    at_full[:KIN, :] = ew.T
    w1p = np.zeros((KIN_PAD, H_MLP), f)
    w1p[:KIN, :] = np.asarray(mlp_w1, f)
    node = np.asarray(node_emb, f)
    w1h_np, w1l_np = split_hl(w1p)
    w2h_np, w2l_np = split_hl(np.asarray(mlp_w2, f))

    shared = {
        "w1h": w1h_np, "w1l": w1l_np, "b1": np.asarray(mlp_b1, f),
        "w2h": w2h_np, "w2l": w2l_np,
        "b2h": np.asarray(mlp_b2, f).astype(np.float16),
        "b2l": (np.asarray(mlp_b2, f) - np.asarray(mlp_b2, f).astype(np.float16).astype(f)).astype(np.float16),
        "cw1": np.ascontiguousarray(np.asarray(conv_w1, f)),
        "b1c": np.asarray(conv_b1, f),
        "cw2": np.ascontiguousarray(np.asarray(conv_w2, f)),
        "b2c": np.asarray(conv_b2, f),
        "gamma": np.asarray(bn_gamma, f), "beta": np.asarray(bn_beta, f),
    }
    in_maps = []
    for c in range(N_CORES):
        sl = slice(c * ROWS, (c + 1) * ROWS)
        m = dict(shared)
        m["at_h"], m["at_l"] = split_hl(at_full[:, sl])
        m["nodet"] = np.ascontiguousarray(node[sl].T)
        in_maps.append(m)

    try:
        res = run_bass_kernel_spmd(nc, in_maps, list(range(N_CORES)), trace=TRACE)
    except Exception:
        # A freshly loaded NEFF occasionally leaves the device wedged
        # (NRT_EXEC_UNIT_UNRECOVERABLE). Running a trivial program clears the
        # state; retry once.
        try:
            _device_reset()
        except Exception:
            pass
        res = run_bass_kernel_spmd(nc, in_maps, list(range(N_CORES)), trace=TRACE)
    LAST_INFO["exec_time_ns"] = res.exec_time_ns
    LAST_INFO["mean_exec_time_ns"] = res.mean_exec_time_ns

    outp = np.empty((N, OUT), f)
    for c in range(N_CORES):
        outp[c * ROWS:(c + 1) * ROWS] = res.results[c]["out"].T
    return outp
